# revision 28
# baseline (speedup 1.0000x reference)
"""
Trainium2 Bass kernel for nn_MultiHeadAttention_74586402062628.

Data-parallel across 8 NeuronCores: one batch element per core.

v3: host-side x transpose (xT bf16 + x8T fp8 DMA'd directly, no on-chip
transpose phase), fp8 DoubleRow Q/K projections (2x PE throughput,
eff. K=256 per matmul), fused z+denominator eviction, fast approximate
reciprocal, GELU straight from PSUM for late s-blocks, split softmax
normalization halves.

Per-core formulation (B=8, S=1000, E=1024, H=16, D=64):
  - xT [E,S] bf16 and x8T [E,S] fp8e4 arrive pre-transposed from host.
  - Q,K projections run in fp8 DoubleRow mode: lhsT packs two 128-row
    weight tiles per PE cell ([128,2,128] fp8), rhs is x8T [128,2,W],
    4 accumulation steps instead of 8. PSUM is evicted on ACT (Copy with
    per-partition bias) to qT,kT [H*D, S] bf16 (head h in 128-row tile
    h//2 at partition base (h%2)*64).
  - V runs bf16 from xT into v slabs [P, hp, tb, par, 65] (64 v columns
    + ones column so the AV matmul emits the softmax denominator as
    PSUM row 64). One DVE eviction per (nt, tb).
  - Attention is computed transposed: scoresT[t,s] = k_t . q_s, exp()
    without max-subtraction (logits are small), causal masking as a 0/1
    band multiply on the diagonal tiles only.
  - Per (hp, sti): zp PSUM rows 0:65 (z + denom) are evicted bf16 in ONE
    DVE op; the denominator row DMAs to dn_g[2hp:2hp+2]; reciprocals use
    the single-instruction DVE approx on f32 (ACT upcast first), cast to
    bf16, and broadcast across partitions with a K=16 selector matmul.
    Normalization is split into head-pair halves so the tail only waits
    on hp 4-7.
  - Output projection accumulates + bp via a K=1 ones matmul; sb 0-3 are
    staged bf16 (their GELUs run right after the last Exp, overlapping
    the trailing p4 matmuls); sb 4-7 GELU directly from PSUM.
  - bv is folded into an effective output bias bpe = bp + bv @ wp.
"""

import math
import os
import sys

for _p in ("/opt/trn_rl_repo", "/opt/pypackages"):
    if _p not in sys.path:
        sys.path.insert(0, _p)

import numpy as np

B, S, E, H, D = 8, 1000, 1024, 16, 64
P = 128
NB = 8                      # 128-row blocks covering S (last is partial)
LAST = S - (NB - 1) * P     # 104
KT = 8                      # 128-row tiles covering E
TP = 4                      # fp8 DoubleRow 256-row contraction pairs
ST = ((0, 512), (512, 488))     # s tiles (start, width) covering S
FT = ((0, 512), (512, 512))     # f/n tiles covering E
SCALE = 1.0 / math.sqrt(S)
NCORES = 8
VM = 65                     # v slab columns: 64 v + 1 ones (denominator)

# BASSMHA_NO_GELU=1: replace final GELU with Identity (CoreSim lacks Gelu)
_NO_GELU = os.environ.get("BASSMHA_NO_GELU", "0") == "1"

_CACHE = {}


def _build_nc():
    from concourse import bacc
    import concourse.mybir as mybir
    from concourse import tile
    from concourse.masks import make_identity

    dt = mybir.dt
    f32 = dt.float32
    bf = dt.bfloat16
    f8 = dt.float8e4
    AF = mybir.ActivationFunctionType
    Alu = mybir.AluOpType
    PM = mybir.MatmulPerfMode

    nc = bacc.Bacc("TRN2", debug=False, target_bir_lowering=False,
                   num_devices=NCORES)

    xT_d = nc.declare_dram_parameter("xTb", [KT, P, S], bf, isOutput=False)
    x8_d = nc.declare_dram_parameter("x8T", [KT, P, S], f8, isOutput=False)
    # wq8[mg, k, i, t, m] = wq2[t*256 + i*128 + k, mg*256 + m]
    # (DoubleRow pairing, contiguous 2KB per partition per mg)
    wq8_d = nc.declare_dram_parameter("wq8", [4, P, 2, TP, 256], f8,
                                      isOutput=False)
    wk8_d = nc.declare_dram_parameter("wk8", [4, P, 2, TP, 256], f8,
                                      isOutput=False)
    # wv3[nt, p, k, n] = wv2[k*128 + p, nt*512 + n]
    wv_d = nc.declare_dram_parameter("wv3", [2, P, KT, 512], bf,
                                     isOutput=False)
    wp_d = nc.declare_dram_parameter("wp3", [2, P, KT, 512], bf,
                                     isOutput=False)
    bq_d = nc.declare_dram_parameter("bqt", [P, KT], f32, isOutput=False)
    bk_d = nc.declare_dram_parameter("bkt", [P, KT], f32, isOutput=False)
    bp_d = nc.declare_dram_parameter("bpe", [1, E], bf, isOutput=False)
    # sel8[k, par*512 + h4*64 + m] = 1 iff k == 2*h4 + par (h4 = hp%4):
    # row-selector for broadcasting packed denominator reciprocals
    sel_d = nc.declare_dram_parameter("sel8", [8, 1024], bf, isOutput=False)
    out_d = nc.declare_dram_parameter("out", [S, E], bf, isOutput=True)

    def g2(ap):
        return ap.rearrange("p (g c) -> p g c", g=2)

    with tile.TileContext(nc) as tc:
        with (
            tc.tile_pool(name="const", bufs=1) as constp,
            tc.tile_pool(name="persist", bufs=1) as persist,
            tc.tile_pool(name="wqk8", bufs=4) as wqk8p,
            tc.tile_pool(name="wv", bufs=2) as wvp,
            tc.tile_pool(name="wp", bufs=2) as wpp,
            tc.tile_pool(name="exp", bufs=5) as expp,
            tc.tile_pool(name="zraw", bufs=8) as zrawp,
            tc.tile_pool(name="dng", bufs=6) as dngp,
            tc.tile_pool(name="dnf", bufs=2) as dnfp,
            tc.tile_pool(name="rpg", bufs=4) as rpgp,
            tc.tile_pool(name="zt", bufs=2) as ztp,
            tc.tile_pool(name="outp", bufs=4) as outp,
            # PSUM budget (8 banks): ps512 2x1 + sp 2x2 + zp 1x2 = 8
            tc.tile_pool(name="ps512", bufs=2, space="PSUM") as ps512,
            tc.tile_pool(name="sp", bufs=2, space="PSUM") as spsum,
            tc.tile_pool(name="zp", bufs=1, space="PSUM") as zpsum,
        ):
            # ---------------- persistent activations ----------------
            # x tiles split by s-phase: st0 work starts on half the data
            xT_a = persist.tile([P, KT, 512], bf)
            xT_b = persist.tile([P, KT, S - 512], bf)
            x8T_a = persist.tile([P, KT, 512], f8)
            x8T_b = persist.tile([P, KT, S - 512], f8)
            xTs = {0: xT_a, 1: xT_b}
            x8s = {0: x8T_a, 1: x8T_b}
            qT = persist.tile([P, KT, S], bf)        # [hd, m, s]
            kT = persist.tile([P, KT, S], bf)
            v = persist.tile([P, H // 2, NB, 2, VM], bf)
            yT = persist.tile([P, KT, S], bf)        # normalized z, stacked
            ostage = persist.tile([P, 4, E], bf)     # pre-GELU staging sb0-3

            # x8T st0 half first (gates the first QK matmuls)
            nc.sync.dma_start(
                x8T_a[:, :, :],
                x8_d[:, :, 0:512].rearrange("k p s -> p k s"))

            # ---------------- constants ----------------
            # msk[r, c] = 1.0 iff c >= r (keep); the causal band mask
            msk = constp.tile([P, P], bf)
            nc.gpsimd.memset(msk[:], 1.0)
            nc.gpsimd.affine_select(
                out=msk[:], in_=msk[:],
                compare_op=Alu.is_ge, fill=0.0,
                base=0, channel_multiplier=-1, pattern=[[1, P]],
            )
            selc = constp.tile([8, 1024], bf)
            nc.sync.dma_start(selc[:], sel_d[:, :])
            ones_b = constp.tile([P, P], bf)     # for bf16 K=1 bias rows
            nc.gpsimd.memset(ones_b[:], 1.0)

            bq_sb = constp.tile([P, KT], f32)
            nc.sync.dma_start(bq_sb[:], bq_d[:, :])
            bk_sb = constp.tile([P, KT], f32)
            nc.sync.dma_start(bk_sb[:], bk_d[:, :])
            bp_sb = constp.tile([1, E], bf)
            nc.sync.dma_start(bp_sb[:], bp_d[:, :])

            # v slab init: zero the tb7 padding rows (t >= 1000 must not
            # contribute), then set the ones column only on valid rows.
            nc.vector.memset(v[96:P, :, NB - 1, :, :], 0.0)
            nc.vector.memset(v[:, :, 0:NB - 1, :, 64:65], 1.0)
            nc.vector.memset(v[0:LAST, :, NB - 1, :, 64:65], 1.0)

            # ---------------- Q/K fp8 DoubleRow projection ----------------
            def emit_qk_w(wd, mg):
                wt = wqk8p.tile([P, 2, TP, 256], f8, tag="w8",
                                name=f"w{mg}_{id(wd)}")
                nc.sync.dma_start(wt[:], wd[mg, :, :, :, :])
                return wt

            def emit_qk1(wt, dst, bias, mg):
                for mi in range(2):
                    m = 2 * mg + mi
                    for sti, (s0, W) in enumerate(ST):
                        ps = ps512.tile([P, 512], f32, tag="b",
                                        name=f"qkps{id(wt)}_{m}_{s0}")
                        for t in range(TP):
                            nc.tensor.matmul(
                                ps[0:P, 0:W],
                                wt[:, :, t, mi * P:(mi + 1) * P],
                                x8s[sti][:, 2 * t:2 * t + 2, 0:W],
                                start=(t == 0), stop=(t == TP - 1),
                                perf_mode=PM.DoubleRow)
                        with nc.allow_low_precision(reason="bf16 evict"):
                            nc.vector.tensor_scalar_add(
                                dst[:, m, s0:s0 + W], ps[0:P, 0:W],
                                bias[:, m:m + 1])

            def emit_qk(mg, wq_wts=None, wk_wts=None):
                emit_qk1(wq_wts or emit_qk_w(wq8_d, mg), qT, bq_sb, mg)
                emit_qk1(wk_wts or emit_qk_w(wk8_d, mg), kT, bk_sb, mg)

            # ---------------- V projection (bf16) ----------------
            def emit_v_weights(nt):
                wv = wvp.tile([P, KT, 512], bf, tag="wv", name=f"wv{nt}")
                nc.sync.dma_start(wv[:], wv_d[nt, :, :, :])
                return wv

            def emit_v(nt, wvs, tbs):
                n0, Wn = FT[nt]
                for tb in tbs:
                    rows = LAST if tb == NB - 1 else P
                    sv = 0 if tb < 4 else 1
                    t0 = tb * P - 512 * sv
                    ps = ps512.tile([P, 512], f32, tag="b",
                                    name=f"vps{nt}_{tb}")
                    for k in range(KT):
                        nc.tensor.matmul(
                            ps[0:rows, 0:Wn],
                            xTs[sv][:, k, t0:t0 + rows],
                            wvs[:, k, 0:Wn],
                            start=(k == 0), stop=(k == KT - 1))
                    src = ps[0:rows, 0:Wn].rearrange(
                        "p (h q e) -> p h q e", q=2, e=64)
                    hp0 = 4 * nt
                    with nc.allow_low_precision(reason="bf16 evict"):
                        nc.vector.tensor_copy(
                            v[0:rows, hp0:hp0 + 4, tb, :, 0:64], src)

            # ---------------- attention emission ----------------
            PIPE = 2
            # Norm groups: head-pairs whose denominators share one packed
            # dn tile + reciprocal batch. sti=1 splits the second half so
            # only hp7's chain trails the last attention.
            GRPS = {0: [[0, 1, 2, 3], [4, 5, 6, 7]],
                    1: [[0, 1, 2, 3], [4, 5], [6], [7]]}
            GRP_OF = {(sti, hp): (gi, g.index(hp))
                      for sti, gs in GRPS.items()
                      for gi, g in enumerate(gs) for hp in g}
            dn_gs = {}

            def emit_dng(sti):
                for gi in range(len(GRPS[sti])):
                    dn_g = dngp.tile([8, 512], f32, tag="dng",
                                     name=f"dng{sti}_{gi}")
                    dn_gs[(sti, gi)] = dn_g

            def emit_attn(hp, sti):
                s0, W = ST[sti]
                n_tb = (s0 + W + P - 1) // P
                zp = zpsum.tile([P, 1024], f32, tag="zp",
                                name=f"zp{hp}_{sti}")
                exs = {}
                geom = {}
                for tb in range(n_tb):
                    rows = LAST if tb == NB - 1 else P
                    t0 = tb * P
                    off = max(0, t0 - s0)
                    geom[tb] = (rows, t0, off, W - off, t0 >= s0)
                for i in range(n_tb + PIPE):
                    if i < n_tb:
                        tb = i
                        rows, t0, off, N, has_diag = geom[tb]
                        sp = spsum.tile([P, 1024], f32, tag="sp",
                                        name=f"sp{hp}_{sti}_{tb}")
                        for par in range(2):
                            base = par * 64
                            nc.tensor.matmul(
                                sp[0:rows, 512 * par:512 * par + N],
                                kT[base:base + 64, hp, t0:t0 + rows],
                                qT[base:base + 64, hp, s0 + off:s0 + W],
                                start=True, stop=True)
                        ex = expp.tile([P, 1024], bf, tag="ex",
                                       name=f"ex{hp}_{sti}_{tb}")
                        exv, spv = g2(ex[:, :]), g2(sp[:, :])
                        if rows < P:
                            nc.vector.memset(exv[96:P, :, 0:N], 0.0)
                        with nc.allow_low_precision(reason="bf16 exp"):
                            nc.scalar.activation(
                                exv[0:rows, :, 0:N], spv[0:rows, :, 0:N],
                                AF.Exp, scale=SCALE)
                        if has_diag:
                            dw = min(P, N)
                            with nc.allow_low_precision(reason="bf16 mask"):
                                nc.gpsimd.tensor_tensor(
                                    exv[0:rows, 0, 0:dw],
                                    exv[0:rows, 0, 0:dw],
                                    msk[0:rows, 0:dw], op=Alu.mult)
                                nc.gpsimd.tensor_tensor(
                                    exv[0:rows, 1, 0:dw],
                                    exv[0:rows, 1, 0:dw],
                                    msk[0:rows, 0:dw], op=Alu.mult)
                        exs[tb] = ex
                    j = i - PIPE
                    if 0 <= j < n_tb:
                        rows, t0, off, N, has_diag = geom[j]
                        ex = exs.pop(j)
                        for par in range(2):
                            nc.tensor.matmul(
                                zp[0:VM, 512 * par + off:512 * par + W],
                                v[:, hp, j, par, 0:VM],
                                ex[0:P, 512 * par:512 * par + N],
                                start=(j == 0), stop=(j == n_tb - 1),
                                skip_group_check=True)
                # one fused eviction: z rows 0:64 + denominator row 64,
                # both column halves -> zraw f32 [65, 2, W]
                zraw = zrawp.tile([P, 1024], f32, tag="zr",
                                  name=f"zr{hp}_{sti}")
                nc.vector.tensor_copy(
                    g2(zraw[:, :])[0:VM, :, 0:W],
                    g2(zp[:, :])[0:VM, :, 0:W])
                # denominator rows (both parities) -> packed dn_g rows
                gi, idx = GRP_OF[(sti, hp)]
                nc.sync.dma_start(
                    dn_gs[(sti, gi)][2 * idx:2 * idx + 2, 0:W],
                    g2(zraw[:, :])[64:65, :, 0:W])
                return zraw

            IDMASK = list(range(32))

            def emit_norm(sti, zraws, hps, fillers=(), shuffle=False,
                          bc_zp=False):
                s0, W = ST[sti]
                gi = GRP_OF[(sti, hps[0])][0]
                nr = 2 * len(GRPS[sti][gi])
                dn_g = dn_gs[(sti, gi)]
                fillers = list(fillers)
                # f32 denominators straight off the zraw evict: one fast
                # DVE reciprocal, then a bf16 downcast for the broadcast
                rp_f = dnfp.tile([8, 512], f32, tag="rpf",
                                 name=f"rpf{sti}_{gi}")
                nc.vector.reciprocal_approx_fast(
                    rp_f[0:nr, 0:W], dn_g[0:nr, 0:W])
                rp_g = rpgp.tile([8, 512], bf, tag="rpg",
                                 name=f"rpg{sti}_{gi}")
                with nc.allow_low_precision(reason="bf16 recip bcast"):
                    nc.vector.tensor_copy(rp_g[0:nr, 0:W],
                                          rp_f[0:nr, 0:W])
                for hp in hps:
                    h4 = GRP_OF[(sti, hp)][1]
                    zraw = zraws[hp]
                    if bc_zp:
                        bcz = zpsum.tile([P, 1024], f32, tag="zp",
                                         name=f"bcz{hp}_{sti}")
                        bc = bcz[:, 0:512]
                    else:
                        bc = ps512.tile([P, 512], f32, tag="b",
                                        name=f"bc{hp}_{sti}")
                    for par in range(2):
                        nc.tensor.matmul(
                            bc[64 * par:64 * par + 64, 0:W],
                            selc[0:nr, 512 * par + h4 * 64:
                                 512 * par + (h4 + 1) * 64],
                            rp_g[0:nr, 0:W],
                            start=True, stop=True, skip_group_check=True)
                    with nc.allow_low_precision(reason="bf16 yT"):
                        nc.vector.tensor_tensor(
                            yT[0:64, hp, s0:s0 + W],
                            g2(zraw[:, :])[0:64, 0, 0:W], bc[0:64, 0:W],
                            op=Alu.mult)
                        # odd heads partition-shift 0:64 -> 64:128 via DMA
                        zt_o = ztp.tile([64, 512], bf, tag="zt",
                                        name=f"zt{hp}_{sti}")
                        nc.vector.tensor_tensor(
                            zt_o[0:64, 0:W],
                            g2(zraw[:, :])[0:64, 1, 0:W], bc[64:P, 0:W],
                            op=Alu.mult)
                        if shuffle:
                            nc.vector.stream_shuffle(
                                yT[64:P, hp, s0:s0 + W], zt_o[0:64, 0:W],
                                IDMASK)
                        else:
                            nc.sync.dma_start(
                                yT[64:P, hp, s0:s0 + W], zt_o[0:64, 0:W])
                    if fillers:
                        fillers.pop(0)()
                for f in fillers:
                    f()

            # ---------------- output projection ----------------
            def emit_wp_loads(fi):
                w = wpp.tile([P, KT, 512], bf, tag="wp", name=f"wp{fi}")
                nc.sync.dma_start(w[:], wp_d[fi, :, :, :])
                return {(fi, k): w[:, k, :] for k in range(KT)}

            p4_open = {}

            def emit_p4a(wps, fi, sb, ps=None):
                # first half-contraction (head-pairs 0-3): only needs yT
                # st0/st1 for hp 0-3, so it can run before the last norm
                f0, Fw = FT[fi]
                rows = LAST if sb == NB - 1 else P
                r0 = sb * P
                if ps is None:
                    ps = ps512.tile([P, 512], f32, tag="b",
                                    name=f"p4a{fi}_{sb}")
                for k in range(4):
                    nc.tensor.matmul(
                        ps[0:rows, 0:Fw],
                        yT[:, k, r0:r0 + rows],
                        wps[(fi, k)][0:P, 0:Fw],
                        start=(k == 0), stop=False, skip_group_check=True)
                p4_open[(fi, sb)] = ps

            def emit_p4b(wps, fi, sb, direct_out=False):
                f0, Fw = FT[fi]
                rows = LAST if sb == NB - 1 else P
                r0 = sb * P
                ps = p4_open.pop((fi, sb))
                for k in range(4, KT):
                    nc.tensor.matmul(
                        ps[0:rows, 0:Fw],
                        yT[:, k, r0:r0 + rows],
                        wps[(fi, k)][0:P, 0:Fw],
                        start=False, stop=False, skip_group_check=True)
                nc.tensor.matmul(
                    ps[0:rows, 0:Fw],
                    ones_b[0:1, 0:rows],
                    bp_sb[0:1, f0:f0 + Fw],
                    start=False, stop=True, skip_group_check=True)
                ot = outp.tile([P, 512], bf, tag="ot", name=f"otb{fi}_{sb}")
                act = AF.Identity if _NO_GELU else AF.Gelu
                with nc.allow_low_precision(reason="bf16 out"):
                    nc.scalar.activation(
                        ot[0:rows, 0:Fw], ps[0:rows, 0:Fw], act)
                nc.gpsimd.dma_start(out_d[r0:r0 + rows, f0:f0 + Fw],
                                    ot[0:rows, 0:Fw])

            def emit_p4(wps, fi, sb, direct_out=False):
                f0, Fw = FT[fi]
                rows = LAST if sb == NB - 1 else P
                r0 = sb * P
                ps = ps512.tile([P, 512], f32, tag="b", name=f"p4{fi}_{sb}")
                for k in range(KT):
                    nc.tensor.matmul(
                        ps[0:rows, 0:Fw],
                        yT[:, k, r0:r0 + rows],
                        wps[(fi, k)][0:P, 0:Fw],
                        start=(k == 0), stop=False)
                # + bias row via K=1 ones matmul
                nc.tensor.matmul(
                    ps[0:rows, 0:Fw],
                    ones_b[0:1, 0:rows],
                    bp_sb[0:1, f0:f0 + Fw],
                    start=False, stop=True)
                if direct_out:
                    # GELU straight from PSUM (all Exps already emitted)
                    ot = outp.tile([P, 512], bf, tag="ot",
                                   name=f"ot{fi}_{sb}")
                    act = AF.Identity if _NO_GELU else AF.Gelu
                    with nc.allow_low_precision(reason="bf16 out"):
                        nc.scalar.activation(
                            ot[0:rows, 0:Fw], ps[0:rows, 0:Fw], act)
                    nc.gpsimd.dma_start(out_d[r0:r0 + rows, f0:f0 + Fw],
                                        ot[0:rows, 0:Fw])
                else:
                    with nc.allow_low_precision(reason="bf16 stage"):
                        nc.vector.tensor_copy(
                            ostage[0:rows, sb, f0:f0 + Fw],
                            ps[0:rows, 0:Fw])

            def emit_out(fi, sb, scale=1.0):
                f0, Fw = FT[fi]
                rows = LAST if sb == NB - 1 else P
                r0 = sb * P
                ot = outp.tile([P, 512], bf, tag="ot", name=f"og{fi}_{sb}")
                act = AF.Identity if _NO_GELU else AF.Gelu
                with nc.allow_low_precision(reason="bf16 out"):
                    nc.scalar.activation(
                        ot[0:rows, 0:Fw], ostage[0:rows, sb, f0:f0 + Fw],
                        act, scale=scale)
                nc.gpsimd.dma_start(out_d[r0:r0 + rows, f0:f0 + Fw],
                                    ot[0:rows, 0:Fw])

            # ---------------- interleaved schedule ----------------
            z0, z1 = {}, {}
            # startup DMA priority: x8T queued above; mg0 weights next
            # (gate the first QK matmul), then xT (gates V), then the rest
            wq_w0 = emit_qk_w(wq8_d, 0)
            wk_w0 = emit_qk_w(wk8_d, 0)
            nc.sync.dma_start(
                x8T_b[:, :, :],
                x8_d[:, :, 512:S].rearrange("k p s -> p k s"))
            wq_w1 = emit_qk_w(wq8_d, 1)
            wk_w1 = emit_qk_w(wk8_d, 1)
            nc.sync.dma_start(
                xT_a[:, :, :], xT_d[:, :, 0:512].rearrange("k p s -> p k s"))
            wv0 = emit_v_weights(0)
            nc.sync.dma_start(
                xT_b[:, :, :], xT_d[:, :, 512:S].rearrange("k p s -> p k s"))
            emit_dng(0)
            emit_qk(0, wq_w0, wk_w0)
            emit_v(0, wv0, range(4))
            emit_qk(1, wq_w1, wk_w1)
            z0[0] = emit_attn(0, 0)
            emit_v(0, wv0, range(4, NB))
            z0[1] = emit_attn(1, 0)
            emit_qk(2)
            z0[2] = emit_attn(2, 0)
            wv1 = emit_v_weights(1)
            emit_v(1, wv1, range(4))
            z0[3] = emit_attn(3, 0)
            emit_qk(3)
            z0[4] = emit_attn(4, 0)
            emit_v(1, wv1, range(4, NB))
            z0[5] = emit_attn(5, 0)
            emit_norm(0, z0, range(4))
            z0[6] = emit_attn(6, 0)
            wps = emit_wp_loads(0)
            z0[7] = emit_attn(7, 0)
            wps.update(emit_wp_loads(1))
            emit_dng(1)
            z1[0] = emit_attn(0, 1)
            emit_norm(0, z0, range(4, 8))
            z1[1] = emit_attn(1, 1)
            emit_p4(wps, 0, 0)
            z1[2] = emit_attn(2, 1)
            emit_p4(wps, 1, 0)
            z1[3] = emit_attn(3, 1)
            emit_p4(wps, 0, 1)
            z1[4] = emit_attn(4, 1)
            emit_norm(1, z1, range(4), fillers=[
                lambda: emit_p4(wps, 1, 1),
                lambda: emit_p4(wps, 0, 2),
            ])
            z1[5] = emit_attn(5, 1)
            emit_p4(wps, 1, 2)
            z1[6] = emit_attn(6, 1)
            emit_p4(wps, 0, 3)
            emit_norm(1, z1, [4, 5])
            emit_norm(1, z1, [6])
            z1[7] = emit_attn(7, 1)
            emit_p4(wps, 1, 3)
            # last exp has been emitted; remaining ACT ops are GELU/Copy.
            # sp-pool banks (attention is done) hold extra half-accumulated
            # p4 tiles as fillers while {7}'s denominator chain settles.
            sp_a = spsum.tile([P, 1024], f32, tag="sp", name="sp_p4a")
            sp_b = spsum.tile([P, 1024], f32, tag="sp", name="sp_p4b")
            emit_norm(1, z1, [7], shuffle=True, bc_zp=True,
                      fillers=[
                          lambda: emit_p4a(wps, 0, 4),
                          lambda: emit_p4a(wps, 1, 4),
                          lambda: emit_p4a(wps, 0, 5, ps=sp_a[:, 0:512]),
                          lambda: emit_p4a(wps, 1, 5, ps=sp_a[:, 512:1024]),
                      ])
            emit_p4a(wps, 0, 6, ps=sp_b[:, 0:512])
            emit_p4a(wps, 1, 6, ps=sp_b[:, 512:1024])
            # sb 0-3 staged GELUs FIRST on the ACT queue (they are ready
            # as soon as the tail begins); their scale operand is a ones
            # column computed from the LAST attention's zraw, so they
            # cannot be hoisted before the last Exp (no table swaps).
            ssc = constp.tile([P, 1], f32)
            nc.vector.memset(ssc[:, 0:1], 1.0)
            nc.vector.tensor_scalar(
                ssc[0:VM, 0:1], z1[7][0:VM, 0:1], 0.0, 1.0,
                op0=Alu.mult, op1=Alu.add)
            for i in range(4):
                emit_out(0, i, scale=ssc[:, 0:1])
                emit_out(1, i, scale=ssc[:, 0:1])
            # sb 4-7: GELU straight from PSUM
            emit_p4b(wps, 0, 4, direct_out=True)
            emit_p4b(wps, 1, 4, direct_out=True)
            emit_p4b(wps, 0, 5, direct_out=True)
            emit_p4b(wps, 1, 5, direct_out=True)
            emit_p4(wps, 0, 7, direct_out=True)
            emit_p4(wps, 1, 7, direct_out=True)
            emit_p4b(wps, 0, 6, direct_out=True)
            emit_p4b(wps, 1, 6, direct_out=True)

    nc.compile()
    return nc


def get_nc():
    if "nc" not in _CACHE:
        _CACHE["nc"] = _build_nc()
    return _CACHE["nc"]


def make_in_maps(inputs):
    import ml_dtypes
    bfnp = ml_dtypes.bfloat16
    f8np = ml_dtypes.float8_e4m3

    x = np.asarray(inputs["x"], np.float32)
    wq = np.asarray(inputs["wq"], np.float32)
    wk = np.asarray(inputs["wk"], np.float32)
    wv = np.asarray(inputs["wv"], np.float32)
    wp = np.asarray(inputs["wp"], np.float32)
    bq = np.asarray(inputs["bq"], np.float32)
    bk = np.asarray(inputs["bk"], np.float32)
    bv = np.asarray(inputs["bv"], np.float32)
    bp = np.asarray(inputs["bp"], np.float32)

    # [H, E, D] -> [E, H*D] (concat head outputs along columns)
    wq2 = wq.transpose(1, 0, 2).reshape(E, E)
    wk2 = wk.transpose(1, 0, 2).reshape(E, E)
    # wv3[nt, p, k, n] = wv2[k*128 + p, nt*512 + n]
    wv3 = np.ascontiguousarray(
        wv.transpose(1, 0, 2).reshape(E, E)
        .reshape(KT, P, 2, 512).transpose(2, 1, 0, 3).astype(bfnp))
    wp3 = np.ascontiguousarray(
        wp.reshape(KT, P, 2, 512).transpose(2, 1, 0, 3).astype(bfnp))
    # fp8 DoubleRow pairing, mg-contiguous:
    # wq8[mg, k, i, t, m] = wq2[t*256 + i*128 + k, mg*256 + m]
    wq8 = np.ascontiguousarray(
        wq2.reshape(TP, 2, P, 4, 256).transpose(3, 2, 1, 0, 4).astype(f8np))
    wk8 = np.ascontiguousarray(
        wk2.reshape(TP, 2, P, 4, 256).transpose(3, 2, 1, 0, 4).astype(f8np))
    # per-partition bias layout: bqt[p, m] = bq_flat[m*128 + p]
    bqt = np.ascontiguousarray(bq.reshape(-1).reshape(KT, P).T)
    bkt = np.ascontiguousarray(bk.reshape(-1).reshape(KT, P).T)
    # fold bv into output bias: y = z + bv  =>  out += bv @ wp
    bpe = (bp.astype(np.float64)
           + bv.reshape(-1).astype(np.float64) @ wp.astype(np.float64))
    bpe = np.ascontiguousarray(
        bpe.astype(np.float32).reshape(1, E).astype(bfnp))

    # sel8[k, par*512 + h4*64 + m] = 1 iff k == 2*h4 + par
    sel8 = np.zeros((8, 1024), np.float32)
    for h4 in range(4):
        for par in range(2):
            sel8[2 * h4 + par,
                 par * 512 + h4 * 64:par * 512 + (h4 + 1) * 64] = 1.0
    sel8 = np.ascontiguousarray(sel8.astype(bfnp))

    shared = {"wq8": wq8, "wk8": wk8, "wv3": wv3, "wp3": wp3,
              "bqt": bqt, "bkt": bkt, "bpe": bpe, "sel8": sel8}
    maps = []
    for b in range(B):
        xTb = x[b].T  # [E, S]
        xT_t = np.ascontiguousarray(
            xTb.reshape(KT, P, S).astype(bfnp))
        x8_t = np.ascontiguousarray(
            xTb.reshape(KT, P, S).astype(f8np))
        maps.append(dict(shared, xTb=xT_t, x8T=x8_t))
    return maps


def run(inputs, trace=False):
    from concourse.bass_utils import run_bass_kernel_spmd
    nc = get_nc()
    in_maps = make_in_maps(inputs)
    res = run_bass_kernel_spmd(nc, in_maps, list(range(NCORES)), trace=trace)
    out = np.stack([np.asarray(res.results[i]["out"]) for i in range(NCORES)])
    return out.astype(np.float32), res


def kernel(**inputs):
    out, _ = run(inputs, trace=False)
    return out
